# revision 1
# baseline (speedup 1.0000x reference)
"""KBC filtered-ranking kernel for 8 Trainium2 NeuronCores.

rank_i = 1 + #{ j unmasked : scores[i,j] >= scores[i, true_i] }

Device (per core, SPMD over column chunks of rhs):
  - scores chunk = q @ rhs_chunk via fp32 PE matmuls ([128,500] tiles,
    K=512 as 4x128 PSUM accumulation)
  - t_i = device score of the true column (diag of a q_block @ gt matmul,
    bit-identical arithmetic to the main matmul) -- computed redundantly
    on every core
  - count_i = #{ j in chunk : s_ij > t_i }  (strict >, so the true column
    self-excludes exactly; fp32 ties elsewhere have ~0 probability)
Host:
  - subtracts the filtered (known-true) tails: for the deduplicated filter
    indices (!= true), count fp64 scores > t and subtract. Only borderline
    |s - t| < 1e-4-ish cells can disagree with the device fp32 count;
    for random filter positions that's ~0.5 cells in 131k, at bulk ranks.
  - ranks = 1 + sum_core counts - corr

Set KBC_F32R=1 to run the matmuls in float32r (1 cycle/row vs fp32's 4,
~4x faster PE): measured L2 rel err 8.4e-05, max per-row rel 6e-3
(reduced-precision PE rounding shifts most ranks by a few units). The
self-exclusion stays exact (t flows through the same f32r pipeline).
Default is full fp32: L2 rel 1.1e-06, max per-row rel 4.6e-05.
"""

import os
from contextlib import ExitStack

import numpy as np

B, D, N = 2048, 512, 100000
NCORES = 8
COLS = N // NCORES          # 12500 columns per core
NTW = 500                   # n-tile width
NT = COLS // NTW            # 25 n-tiles per core
NB = B // 128               # 16 row blocks
KT = D // 128               # 4 k tiles
P = 128

_CACHE = {}


F32R = os.environ.get("KBC_F32R", "0") == "1"


def _gen():
    import concourse.bass as bass
    import concourse.mybir as mybir

    mdt = mybir.dt.float32r if F32R else mybir.dt.float32
    nc = bass.Bass()
    qT_d = nc.dram_tensor("qT", [P, KT, B], mdt, kind="ExternalInput")
    rhs_d = nc.dram_tensor(
        "rhsc", [NT, P, KT, NTW], mdt, kind="ExternalInput"
    )
    gt_d = nc.dram_tensor("gt", [P, KT, B], mdt, kind="ExternalInput")
    eye_d = nc.dram_tensor("eye", [P, P], mybir.dt.float32, kind="ExternalInput")
    cnt_d = nc.dram_tensor("cnt", [P, NB], mybir.dt.float32, kind="ExternalOutput")
    tv_d = nc.dram_tensor("tv", [P, NB], mybir.dt.float32, kind="ExternalOutput")

    ge = mybir.AluOpType
    with ExitStack() as ctx:
        tq = ctx.enter_context(nc.sbuf_tensor([P, KT, B], mdt))
        tg = ctx.enter_context(nc.sbuf_tensor([P, KT, B], mdt))
        teye = ctx.enter_context(nc.sbuf_tensor([P, P], mybir.dt.float32))
        trh = ctx.enter_context(nc.sbuf_tensor([P, 3, KT, NTW], mdt))
        tall = ctx.enter_context(nc.sbuf_tensor([P, NB], mybir.dt.float32))
        acc = ctx.enter_context(nc.sbuf_tensor([P, NB, NT], mybir.dt.float32))
        cnt = ctx.enter_context(nc.sbuf_tensor([P, NB], mybir.dt.float32))
        dscr = ctx.enter_context(nc.sbuf_tensor([P, P], mybir.dt.float32))
        cscr = ctx.enter_context(nc.sbuf_tensor([P, NTW], mybir.dt.bfloat16))
        psm = ctx.enter_context(nc.psum_tensor([P, 4, 512], mybir.dt.float32))
        pst = ctx.enter_context(nc.psum_tensor([P, 2, 512], mybir.dt.float32))

        dma_q = ctx.enter_context(nc.semaphore())
        dma_r = ctx.enter_context(nc.semaphore())
        mm_sem = ctx.enter_context(nc.semaphore())
        t_sem = ctx.enter_context(nc.semaphore())
        c_sem = ctx.enter_context(nc.semaphore())
        block = ctx.enter_context(nc.Block())

        @block.sync
        def _(sync):
            sync.dma_start(tq[:], qT_d[:]).then_inc(dma_q, 16)
            sync.dma_start(tg[:], gt_d[:]).then_inc(dma_q, 16)
            sync.dma_start(teye[:], eye_d[:]).then_inc(dma_q, 16)
            for nt in range(NT):
                if nt >= 3:
                    # PE finished all blocks of tile nt-3 -> buffer free
                    sync.wait_ge(mm_sem, NB + (nt - 2) * NB)
                sync.dma_start(trh[:, nt % 3], rhs_d[nt]).then_inc(dma_r, 16)
            sync.wait_ge(t_sem, NB)
            sync.dma_start(tv_d[:], tall[:]).then_inc(dma_q, 16)
            sync.wait_ge(c_sem, NB * NT + 1)
            sync.dma_start(cnt_d[:], cnt[:]).then_inc(dma_q, 16)

        @block.tensor
        def _(tensor):
            tensor.wait_ge(dma_q, 48)
            # t-phase: true-column scores, one [128,128] tile per block
            for b in range(NB):
                if b >= 2:
                    tensor.wait_ge(t_sem, b - 1)
                for k in range(KT):
                    mm = nc.tensor.matmul(
                        pst[:, b % 2, 0:P],
                        tq[:, k, b * P : (b + 1) * P],
                        tg[:, k, b * P : (b + 1) * P],
                        start=(k == 0),
                        stop=(k == KT - 1),
                    )
                    if k == KT - 1:
                        mm.then_inc(mm_sem, 1)
            # main loop
            for nt in range(NT):
                tensor.wait_ge(dma_r, (nt + 1) * 16)
                for b in range(NB):
                    i = nt * NB + b
                    if i >= 4:
                        tensor.wait_ge(c_sem, i - 3)
                    for k in range(KT):
                        mm = nc.tensor.matmul(
                            psm[:, i % 4, 0:NTW],
                            tq[:, k, b * P : (b + 1) * P],
                            trh[:, nt % 3, k],
                            start=(k == 0),
                            stop=(k == KT - 1),
                        )
                        if k == KT - 1:
                            mm.then_inc(mm_sem, 1)

        @block.vector
        def _(vector):
            vector.wait_ge(dma_q, 48)
            for b in range(NB):
                vector.wait_ge(mm_sem, b + 1)
                nc.vector.tensor_copy(dscr[:], pst[:, b % 2, 0:P])
                nc.vector.scalar_tensor_tensor(
                    out=dscr[:],
                    in0=dscr[:],
                    scalar=1.0,
                    in1=teye[:],
                    op0=ge.mult,
                    op1=ge.mult,
                    accum_out=tall[:, b : b + 1],
                ).then_inc(t_sem, 1)
            for nt in range(NT):
                for b in range(NB):
                    i = nt * NB + b
                    vector.wait_ge(mm_sem, NB + i + 1)
                    nc.vector.tensor_scalar(
                        cscr[:],
                        psm[:, i % 4, 0:NTW],
                        tall[:, b : b + 1],
                        0.0,
                        op0=ge.is_gt,
                        op1=ge.add,
                        accum_out=acc[:, b, nt : nt + 1],
                    ).then_inc(c_sem, 1)
            for b in range(NB):
                red = nc.vector.tensor_reduce(
                    cnt[:, b : b + 1],
                    acc[:, b],
                    axis=mybir.AxisListType.X,
                    op=ge.add,
                )
                if b == NB - 1:
                    red.then_inc(c_sem, 1)

    return nc


def _build():
    if "nc" not in _CACHE:
        import concourse.mybir as mybir

        _CACHE["mybir"] = mybir
        _CACHE["nc"] = _gen()
    return _CACHE["nc"]


def _run_pjrt(nc, in_maps, n_cores, reps=0):
    """Mirror of bass2jax.run_bass_via_pjrt with device-resident inputs and
    optional repeat timing (no donation so buffers can be reused)."""
    import time as _time

    import jax
    from jax.sharding import Mesh, NamedSharding, PartitionSpec

    try:
        from jax.experimental.shard_map import shard_map
    except ImportError:  # newer jax
        from jax.shard_map import shard_map

    import concourse.mybir as mybir
    from concourse import bass2jax

    bass2jax.install_neuronx_cc_hook()
    partition_name = nc.partition_id_tensor.name if nc.partition_id_tensor else None
    in_names, out_names, out_avals, zero_outs = [], [], [], []
    for alloc in nc.m.functions[0].allocations:
        if not isinstance(alloc, mybir.MemoryLocationSet):
            continue
        name = alloc.memorylocations[0].name
        if alloc.kind == "ExternalInput":
            if name != partition_name:
                in_names.append(name)
        elif alloc.kind == "ExternalOutput":
            out_names.append(name)
            shape = tuple(alloc.tensor_shape)
            dtype = mybir.dt.np(alloc.dtype)
            out_avals.append(jax.core.ShapedArray(shape, dtype))
            zero_outs.append(np.zeros(shape, dtype))
    n_params = len(in_names)
    names_all = in_names + out_names + ([partition_name] if partition_name else [])

    def _body(*args):
        operands = list(args)
        if partition_name:
            operands.append(bass2jax.partition_id_tensor())
        outs = bass2jax._bass_exec_p.bind(
            *operands,
            out_avals=tuple(out_avals),
            in_names=tuple(names_all),
            out_names=tuple(out_names),
            lowering_input_output_aliases=(),
            sim_require_finite=True,
            sim_require_nnan=True,
            nc=nc,
        )
        return tuple(outs)

    devices = jax.devices()[:n_cores]
    mesh = Mesh(np.asarray(devices), ("core",))
    in_specs = (PartitionSpec("core"),) * (n_params + len(out_names))
    out_specs = (PartitionSpec("core"),) * len(out_names)
    fn = jax.jit(
        shard_map(
            _body, mesh=mesh, in_specs=in_specs, out_specs=out_specs, check_rep=False
        ),
        keep_unused=True,
    )
    concat_in = [
        np.concatenate([np.asarray(in_maps[c][nm]) for c in range(n_cores)], axis=0)
        for nm in in_names
    ]
    concat_zeros = [
        np.zeros((n_cores * z.shape[0], *z.shape[1:]), z.dtype) for z in zero_outs
    ]
    sh = NamedSharding(mesh, PartitionSpec("core"))
    dev_in = [jax.device_put(x, sh) for x in concat_in]
    dev_zero = [jax.device_put(x, sh) for x in concat_zeros]
    out = fn(*dev_in, *dev_zero)
    jax.block_until_ready(out)
    times = []
    for _ in range(reps):
        t0 = _time.perf_counter()
        o = fn(*dev_in, *dev_zero)
        jax.block_until_ready(o)
        times.append(_time.perf_counter() - t0)
    results = [
        {
            name: np.asarray(out[i]).reshape(n_cores, *out_avals[i].shape)[c]
            for i, name in enumerate(out_names)
        }
        for c in range(n_cores)
    ]
    return results, (min(times) if times else None)


def _run_device(qT, rhs, gt, eye, trace=False, reps=0):
    nc = _build()
    in_maps = []
    for c in range(NCORES):
        in_maps.append(
            {
                "qT": qT,
                "rhsc": np.ascontiguousarray(
                    rhs[:, c * COLS : (c + 1) * COLS]
                    .reshape(KT, P, NT, NTW)
                    .transpose(2, 1, 0, 3)
                ),
                "gt": gt,
                "eye": eye,
            }
        )
    return _run_pjrt(nc, in_maps, NCORES, reps=reps)


def kernel(q, rhs, queries, filter_idx, _trace=False, _ret_exec=False, _reps=0):
    q = np.asarray(q, dtype=np.float32)
    rhs = np.asarray(rhs, dtype=np.float32)
    true_rhs = np.asarray(queries)[:, 2].astype(np.int64)
    filt = np.asarray(filter_idx).astype(np.int64)

    qT = np.ascontiguousarray(q.T.reshape(KT, P, B).transpose(1, 0, 2))
    gt = np.ascontiguousarray(rhs[:, true_rhs].reshape(KT, P, B).transpose(1, 0, 2))
    eye = np.eye(P, dtype=np.float32)

    results, exec_s = _run_device(qT, rhs, gt, eye, reps=_reps)

    counts = np.zeros(B, dtype=np.float64)
    for c in range(NCORES):
        cc = results[c]["cnt"]  # [P, NB]
        counts += cc.T.reshape(B)  # row b*128+p = cc[p, b]
    t = results[0]["tv"].T.reshape(B).astype(np.float32)  # device true scores

    # host correction: dedupe filter, drop entries equal to true tail
    q64 = q.astype(np.float64)
    corr = np.zeros(B, dtype=np.float64)
    CH = 256
    for s in range(0, B, CH):
        e = s + CH
        idx = filt[s:e]  # [CH, 64]
        cols = rhs[:, idx.reshape(-1)].astype(np.float64)  # [512, CH*64]
        sc = np.einsum(
            "bd,dbf->bf", q64[s:e], cols.reshape(D, e - s, idx.shape[1])
        )  # [CH, 64]
        gtmask = sc > t[s:e, None].astype(np.float64)
        # dedupe within row + exclude true index
        srt = np.sort(idx, axis=1)
        first = np.ones_like(idx, dtype=bool)
        order = np.argsort(idx, axis=1, kind="stable")
        dup = srt[:, 1:] == srt[:, :-1]
        fsorted = np.ones_like(idx, dtype=bool)
        fsorted[:, 1:] = ~dup
        np.put_along_axis(first, order, fsorted, axis=1)
        valid = first & (idx != true_rhs[s:e, None])
        corr[s:e] = (gtmask & valid).sum(axis=1)

    ranks = 1.0 + counts - corr
    ranks = np.maximum(ranks, 1.0).astype(np.float32)
    if _ret_exec:
        return ranks, exec_s
    return ranks



# revision 2
# speedup vs baseline: 3.1066x; 3.1066x over previous
"""KBC filtered-ranking kernel for 8 Trainium2 NeuronCores.

rank_i = 1 + #{ j unmasked : scores[i,j] >= scores[i, true_i] }

Device (per core, SPMD over column chunks of rhs):
  - scores chunk = q @ rhs_chunk via fp32 PE matmuls ([128,500] tiles,
    K=512 as 4x128 PSUM accumulation)
  - t_i = device score of the true column (diag of a q_block @ gt matmul,
    bit-identical arithmetic to the main matmul) -- computed redundantly
    on every core
  - count_i = #{ j in chunk : s_ij > t_i }  (strict >, so the true column
    self-excludes exactly; fp32 ties elsewhere have ~0 probability)
Host:
  - subtracts the filtered (known-true) tails: for the deduplicated filter
    indices (!= true), count fp64 scores > t and subtract. Only borderline
    |s - t| < 1e-4-ish cells can disagree with the device fp32 count;
    for random filter positions that's ~0.5 cells in 131k, at bulk ranks.
  - ranks = 1 + sum_core counts - corr

Set KBC_F32R=1 to run the matmuls in float32r (1 cycle/row vs fp32's 4,
~4x faster PE): measured L2 rel err 8.4e-05, max per-row rel 6e-3
(reduced-precision PE rounding shifts most ranks by a few units). The
self-exclusion stays exact (t flows through the same f32r pipeline).
Default is full fp32: L2 rel 1.1e-06, max per-row rel 4.6e-05.
"""

import os
from contextlib import ExitStack

import numpy as np

B, D, N = 2048, 512, 100000
NCORES = 8
COLS = N // NCORES          # 12500 columns per core
NTW = 500                   # n-tile width
NT = COLS // NTW            # 25 n-tiles per core
NB = B // 128               # 16 row blocks
KT = D // 128               # 4 k tiles
P = 128

_CACHE = {}


F32R = os.environ.get("KBC_F32R", "1") == "1"


def _gen():
    import concourse.bass as bass
    import concourse.mybir as mybir

    mdt = mybir.dt.float32r if F32R else mybir.dt.float32
    nc = bass.Bass()
    qT_d = nc.dram_tensor("qT", [P, KT, B], mdt, kind="ExternalInput")
    rhs_d = nc.dram_tensor(
        "rhsc", [NT, P, KT, NTW], mdt, kind="ExternalInput"
    )
    gt_d = nc.dram_tensor("gt", [P, KT, B], mdt, kind="ExternalInput")
    eye_d = nc.dram_tensor("eye", [P, P], mybir.dt.float32, kind="ExternalInput")
    cnt_d = nc.dram_tensor("cnt", [P, NB], mybir.dt.float32, kind="ExternalOutput")
    tv_d = nc.dram_tensor("tv", [P, NB], mybir.dt.float32, kind="ExternalOutput")

    ge = mybir.AluOpType
    with ExitStack() as ctx:
        tq = ctx.enter_context(nc.sbuf_tensor([P, KT, B], mdt))
        tg = ctx.enter_context(nc.sbuf_tensor([P, KT, B], mdt))
        teye = ctx.enter_context(nc.sbuf_tensor([P, P], mybir.dt.float32))
        trh = ctx.enter_context(nc.sbuf_tensor([P, 3, KT, NTW], mdt))
        tall = ctx.enter_context(nc.sbuf_tensor([P, NB], mybir.dt.float32))
        acc = ctx.enter_context(nc.sbuf_tensor([P, NB, NT], mybir.dt.float32))
        cnt = ctx.enter_context(nc.sbuf_tensor([P, NB], mybir.dt.float32))
        dscr = ctx.enter_context(nc.sbuf_tensor([P, P], mybir.dt.float32))
        cscr = ctx.enter_context(nc.sbuf_tensor([P, NTW], mybir.dt.bfloat16))
        psm = ctx.enter_context(nc.psum_tensor([P, 4, 512], mybir.dt.float32))
        pst = ctx.enter_context(nc.psum_tensor([P, 2, 512], mybir.dt.float32))

        dma_q = ctx.enter_context(nc.semaphore())
        dma_r = ctx.enter_context(nc.semaphore())
        mm_sem = ctx.enter_context(nc.semaphore())
        t_sem = ctx.enter_context(nc.semaphore())
        c_sem = ctx.enter_context(nc.semaphore())
        block = ctx.enter_context(nc.Block())

        @block.sync
        def _(sync):
            sync.dma_start(tq[:], qT_d[:]).then_inc(dma_q, 16)
            sync.dma_start(tg[:], gt_d[:]).then_inc(dma_q, 16)
            sync.dma_start(teye[:], eye_d[:]).then_inc(dma_q, 16)
            for nt in range(NT):
                if nt >= 3:
                    # PE finished all blocks of tile nt-3 -> buffer free
                    sync.wait_ge(mm_sem, NB + (nt - 2) * NB)
                sync.dma_start(trh[:, nt % 3], rhs_d[nt]).then_inc(dma_r, 16)
            sync.wait_ge(t_sem, NB)
            sync.dma_start(tv_d[:], tall[:]).then_inc(dma_q, 16)
            sync.wait_ge(c_sem, NB * NT + 1)
            sync.dma_start(cnt_d[:], cnt[:]).then_inc(dma_q, 16)

        @block.tensor
        def _(tensor):
            tensor.wait_ge(dma_q, 48)
            # t-phase: true-column scores, one [128,128] tile per block
            for b in range(NB):
                if b >= 2:
                    tensor.wait_ge(t_sem, b - 1)
                for k in range(KT):
                    mm = nc.tensor.matmul(
                        pst[:, b % 2, 0:P],
                        tq[:, k, b * P : (b + 1) * P],
                        tg[:, k, b * P : (b + 1) * P],
                        start=(k == 0),
                        stop=(k == KT - 1),
                    )
                    if k == KT - 1:
                        mm.then_inc(mm_sem, 1)
            # main loop
            for nt in range(NT):
                tensor.wait_ge(dma_r, (nt + 1) * 16)
                for b in range(NB):
                    i = nt * NB + b
                    if i >= 4:
                        tensor.wait_ge(c_sem, i - 3)
                    for k in range(KT):
                        mm = nc.tensor.matmul(
                            psm[:, i % 4, 0:NTW],
                            tq[:, k, b * P : (b + 1) * P],
                            trh[:, nt % 3, k],
                            start=(k == 0),
                            stop=(k == KT - 1),
                        )
                        if k == KT - 1:
                            mm.then_inc(mm_sem, 1)

        @block.vector
        def _(vector):
            vector.wait_ge(dma_q, 48)
            for b in range(NB):
                vector.wait_ge(mm_sem, b + 1)
                nc.vector.tensor_copy(dscr[:], pst[:, b % 2, 0:P])
                nc.vector.scalar_tensor_tensor(
                    out=dscr[:],
                    in0=dscr[:],
                    scalar=1.0,
                    in1=teye[:],
                    op0=ge.mult,
                    op1=ge.mult,
                    accum_out=tall[:, b : b + 1],
                ).then_inc(t_sem, 1)
            for nt in range(NT):
                for b in range(NB):
                    i = nt * NB + b
                    vector.wait_ge(mm_sem, NB + i + 1)
                    nc.vector.tensor_scalar(
                        cscr[:],
                        psm[:, i % 4, 0:NTW],
                        tall[:, b : b + 1],
                        0.0,
                        op0=ge.is_gt,
                        op1=ge.add,
                        accum_out=acc[:, b, nt : nt + 1],
                    ).then_inc(c_sem, 1)
            for b in range(NB):
                red = nc.vector.tensor_reduce(
                    cnt[:, b : b + 1],
                    acc[:, b],
                    axis=mybir.AxisListType.X,
                    op=ge.add,
                )
                if b == NB - 1:
                    red.then_inc(c_sem, 1)

    return nc


def _build():
    if "nc" not in _CACHE:
        import concourse.mybir as mybir

        _CACHE["mybir"] = mybir
        _CACHE["nc"] = _gen()
    return _CACHE["nc"]


def _run_pjrt(nc, in_maps, n_cores, reps=0):
    """Mirror of bass2jax.run_bass_via_pjrt with device-resident inputs and
    optional repeat timing (no donation so buffers can be reused)."""
    import time as _time

    import jax
    from jax.sharding import Mesh, NamedSharding, PartitionSpec

    try:
        from jax.experimental.shard_map import shard_map
    except ImportError:  # newer jax
        from jax.shard_map import shard_map

    import concourse.mybir as mybir
    from concourse import bass2jax

    bass2jax.install_neuronx_cc_hook()
    partition_name = nc.partition_id_tensor.name if nc.partition_id_tensor else None
    in_names, out_names, out_avals, zero_outs = [], [], [], []
    for alloc in nc.m.functions[0].allocations:
        if not isinstance(alloc, mybir.MemoryLocationSet):
            continue
        name = alloc.memorylocations[0].name
        if alloc.kind == "ExternalInput":
            if name != partition_name:
                in_names.append(name)
        elif alloc.kind == "ExternalOutput":
            out_names.append(name)
            shape = tuple(alloc.tensor_shape)
            dtype = mybir.dt.np(alloc.dtype)
            out_avals.append(jax.core.ShapedArray(shape, dtype))
            zero_outs.append(np.zeros(shape, dtype))
    n_params = len(in_names)
    names_all = in_names + out_names + ([partition_name] if partition_name else [])

    def _body(*args):
        operands = list(args)
        if partition_name:
            operands.append(bass2jax.partition_id_tensor())
        outs = bass2jax._bass_exec_p.bind(
            *operands,
            out_avals=tuple(out_avals),
            in_names=tuple(names_all),
            out_names=tuple(out_names),
            lowering_input_output_aliases=(),
            sim_require_finite=True,
            sim_require_nnan=True,
            nc=nc,
        )
        return tuple(outs)

    devices = jax.devices()[:n_cores]
    mesh = Mesh(np.asarray(devices), ("core",))
    in_specs = (PartitionSpec("core"),) * (n_params + len(out_names))
    out_specs = (PartitionSpec("core"),) * len(out_names)
    fn = jax.jit(
        shard_map(
            _body, mesh=mesh, in_specs=in_specs, out_specs=out_specs, check_rep=False
        ),
        keep_unused=True,
    )
    concat_in = [
        np.concatenate([np.asarray(in_maps[c][nm]) for c in range(n_cores)], axis=0)
        for nm in in_names
    ]
    concat_zeros = [
        np.zeros((n_cores * z.shape[0], *z.shape[1:]), z.dtype) for z in zero_outs
    ]
    sh = NamedSharding(mesh, PartitionSpec("core"))
    dev_in = [jax.device_put(x, sh) for x in concat_in]
    dev_zero = [jax.device_put(x, sh) for x in concat_zeros]
    out = fn(*dev_in, *dev_zero)
    jax.block_until_ready(out)
    times = []
    for _ in range(reps):
        t0 = _time.perf_counter()
        o = fn(*dev_in, *dev_zero)
        jax.block_until_ready(o)
        times.append(_time.perf_counter() - t0)
    results = [
        {
            name: np.asarray(out[i]).reshape(n_cores, *out_avals[i].shape)[c]
            for i, name in enumerate(out_names)
        }
        for c in range(n_cores)
    ]
    return results, (min(times) if times else None)


def _run_device(qT, rhs, gt, eye, trace=False, reps=0):
    nc = _build()
    in_maps = []
    for c in range(NCORES):
        in_maps.append(
            {
                "qT": qT,
                "rhsc": np.ascontiguousarray(
                    rhs[:, c * COLS : (c + 1) * COLS]
                    .reshape(KT, P, NT, NTW)
                    .transpose(2, 1, 0, 3)
                ),
                "gt": gt,
                "eye": eye,
            }
        )
    return _run_pjrt(nc, in_maps, NCORES, reps=reps)


def kernel(q, rhs, queries, filter_idx, _trace=False, _ret_exec=False, _reps=0):
    q = np.asarray(q, dtype=np.float32)
    rhs = np.asarray(rhs, dtype=np.float32)
    true_rhs = np.asarray(queries)[:, 2].astype(np.int64)
    filt = np.asarray(filter_idx).astype(np.int64)

    qT = np.ascontiguousarray(q.T.reshape(KT, P, B).transpose(1, 0, 2))
    gt = np.ascontiguousarray(rhs[:, true_rhs].reshape(KT, P, B).transpose(1, 0, 2))
    eye = np.eye(P, dtype=np.float32)

    results, exec_s = _run_device(qT, rhs, gt, eye, reps=_reps)

    counts = np.zeros(B, dtype=np.float64)
    for c in range(NCORES):
        cc = results[c]["cnt"]  # [P, NB]
        counts += cc.T.reshape(B)  # row b*128+p = cc[p, b]
    t = results[0]["tv"].T.reshape(B).astype(np.float32)  # device true scores

    # host correction: dedupe filter, drop entries equal to true tail
    q64 = q.astype(np.float64)
    corr = np.zeros(B, dtype=np.float64)
    CH = 256
    for s in range(0, B, CH):
        e = s + CH
        idx = filt[s:e]  # [CH, 64]
        cols = rhs[:, idx.reshape(-1)].astype(np.float64)  # [512, CH*64]
        sc = np.einsum(
            "bd,dbf->bf", q64[s:e], cols.reshape(D, e - s, idx.shape[1])
        )  # [CH, 64]
        gtmask = sc > t[s:e, None].astype(np.float64)
        # dedupe within row + exclude true index
        srt = np.sort(idx, axis=1)
        first = np.ones_like(idx, dtype=bool)
        order = np.argsort(idx, axis=1, kind="stable")
        dup = srt[:, 1:] == srt[:, :-1]
        fsorted = np.ones_like(idx, dtype=bool)
        fsorted[:, 1:] = ~dup
        np.put_along_axis(first, order, fsorted, axis=1)
        valid = first & (idx != true_rhs[s:e, None])
        corr[s:e] = (gtmask & valid).sum(axis=1)

    ranks = 1.0 + counts - corr
    ranks = np.maximum(ranks, 1.0).astype(np.float32)
    if _ret_exec:
        return ranks, exec_s
    return ranks



# revision 3
# speedup vs baseline: 5.5567x; 1.7886x over previous
"""KBC filtered-ranking kernel for 8 Trainium2 NeuronCores.

rank_i = 1 + #{ j unmasked : scores[i,j] >= scores[i, true_i] }

Device (per core, SPMD over column chunks of rhs):
  - scores chunk = q @ rhs_chunk via float32r PE matmuls ([128,500] tiles,
    K=512 as 4x128 PSUM accumulation); f32r streams 1 row/cycle on the PE
    (4x fp32) at tf32-like precision: measured rank L2 rel err 8.4e-05.
  - t_i = device score of the true column (diag of a q_block @ gt matmul,
    bit-identical arithmetic to the main matmul) -- computed redundantly
    on every core
  - count_i = #{ j in chunk : s_ij > t_i }  (strict >, so the true column
    self-excludes exactly; fp32 ties elsewhere have ~0 probability)
Host:
  - subtracts the filtered (known-true) tails: for the deduplicated filter
    indices (!= true), count fp64 scores > t and subtract. Only borderline
    |s - t| cells can disagree with the device f32r count; for random
    filter positions that's ~0 cells in 131k, at bulk ranks.
  - ranks = 1 + sum_core counts - corr

Schedule: DMAs are split across both HWDGE queues (SP + Activation) --
qT/gt interleaved per 128-row block so the t-phase starts ~immediately,
then rhs tiles alternate queues. The main loop cycles through all 8 PSUM
banks (t-phase shares banks 0-1 before the main loop reaches them), so
the PE never waits on the DVE count ops. Steady state is PE-bound at
~1.05 cycles/row.

Set KBC_FP32=1 for full-fp32 matmuls (4x slower PE, rank L2 rel 1.1e-06)
if f32r precision is ever insufficient.
"""

import os
from contextlib import ExitStack

import numpy as np

B, D, N = 2048, 512, 100000
NCORES = 8
COLS = N // NCORES          # 12500 columns per core
NTW = 500                   # n-tile width
NT = COLS // NTW            # 25 n-tiles per core
NB = B // 128               # 16 row blocks
KT = D // 128               # 4 k tiles
P = 128
NTB = 4                     # rhs tile buffers in SBUF
PSB = 8                     # main-loop PSUM banks

_CACHE = {}


F32R = os.environ.get("KBC_FP32", "0") != "1"


def _gen():
    import concourse.bass as bass
    import concourse.mybir as mybir

    mdt = mybir.dt.float32r if F32R else mybir.dt.float32
    nc = bass.Bass()
    qT_d = nc.dram_tensor("qT", [P, NB, KT, P], mdt, kind="ExternalInput")
    rhs_d = nc.dram_tensor(
        "rhsc", [NT, P, KT, NTW], mdt, kind="ExternalInput"
    )
    gt_d = nc.dram_tensor("gt", [P, NB, KT, P], mdt, kind="ExternalInput")
    eye_d = nc.dram_tensor("eye", [P, P], mybir.dt.float32, kind="ExternalInput")
    cnt_d = nc.dram_tensor("cnt", [P, NB], mybir.dt.float32, kind="ExternalOutput")
    tv_d = nc.dram_tensor("tv", [P, NB], mybir.dt.float32, kind="ExternalOutput")

    ge = mybir.AluOpType
    with ExitStack() as ctx:
        tq = ctx.enter_context(nc.sbuf_tensor([P, NB, KT, P], mdt))
        tg = ctx.enter_context(nc.sbuf_tensor([P, NB, KT, P], mdt))
        teye = ctx.enter_context(nc.sbuf_tensor([P, P], mybir.dt.float32))
        trh = ctx.enter_context(nc.sbuf_tensor([P, NTB, KT, NTW], mdt))
        tall = ctx.enter_context(nc.sbuf_tensor([P, NB], mybir.dt.float32))
        acc = ctx.enter_context(nc.sbuf_tensor([P, NB, NT], mybir.dt.float32))
        cnt = ctx.enter_context(nc.sbuf_tensor([P, NB], mybir.dt.float32))
        dscr = ctx.enter_context(nc.sbuf_tensor([P, P], mybir.dt.float32))
        cscr = ctx.enter_context(nc.sbuf_tensor([P, NTW], mybir.dt.bfloat16))
        psm = ctx.enter_context(nc.psum_tensor([P, PSB, 512], mybir.dt.float32))

        dsp = ctx.enter_context(nc.semaphore())   # SP-queue DMA completions
        dact = ctx.enter_context(nc.semaphore())  # Act-queue DMA completions
        mm_sem = ctx.enter_context(nc.semaphore())
        t_sem = ctx.enter_context(nc.semaphore())
        c_sem = ctx.enter_context(nc.semaphore())
        block = ctx.enter_context(nc.Block(no_gpsimd_drain=True))

        # Per-queue completion counts (x16 per transfer):
        #   queue(b) chunks: qc_b is transfer #(b+1), gc_b #(b+2) on its
        #   queue (Act leads with eye, giving odd b the same formula).
        #   SP tiles (even nt): #(17 + nt//2); Act tiles (odd): #(18+(nt-1)//2)

        @block.sync
        def _(sync):
            for b in range(0, NB, 2):
                sync.dma_start(tq[:, b], qT_d[:, b]).then_inc(dsp, 16)
                sync.dma_start(tg[:, b], gt_d[:, b]).then_inc(dsp, 16)
            for nt in range(0, NT, 2):
                if nt >= NTB:
                    # PE finished all blocks of tile nt-NTB -> buffer free
                    sync.wait_ge(mm_sem, NB + (nt - NTB + 1) * NB)
                sync.dma_start(trh[:, nt % NTB], rhs_d[nt]).then_inc(dsp, 16)
            sync.wait_ge(t_sem, NB)
            sync.dma_start(tv_d[:], tall[:]).then_inc(dsp, 16)
            sync.wait_ge(c_sem, NB * NT + 1)
            sync.dma_start(cnt_d[:], cnt[:]).then_inc(dsp, 16)

        @block.scalar
        def _(scalar):
            scalar.dma_start(teye[:], eye_d[:]).then_inc(dact, 16)
            for b in range(1, NB, 2):
                scalar.dma_start(tq[:, b], qT_d[:, b]).then_inc(dact, 16)
                scalar.dma_start(tg[:, b], gt_d[:, b]).then_inc(dact, 16)
            for nt in range(1, NT, 2):
                if nt >= NTB:
                    scalar.wait_ge(mm_sem, NB + (nt - NTB + 1) * NB)
                scalar.dma_start(trh[:, nt % NTB], rhs_d[nt]).then_inc(dact, 16)

        @block.tensor
        def _(tensor):
            # t-phase: true-column scores, one [128,128] tile per block,
            # tracking the interleaved qc/gc chunk arrivals
            for b in range(NB):
                if b % 2 == 0:
                    tensor.wait_ge(dsp, 16 * (b + 2))
                else:
                    tensor.wait_ge(dact, 16 * (b + 2))
                if b >= 2:
                    tensor.wait_ge(t_sem, b - 1)
                for k in range(KT):
                    mm = nc.tensor.matmul(
                        psm[:, b % 2, 0:P],
                        tq[:, b, k],
                        tg[:, b, k],
                        start=(k == 0),
                        stop=(k == KT - 1),
                    )
                    if k == KT - 1:
                        mm.then_inc(mm_sem, 1)
            # main loop
            for nt in range(NT):
                if nt % 2 == 0:
                    tensor.wait_ge(dsp, 16 * (17 + nt // 2))
                else:
                    tensor.wait_ge(dact, 16 * (18 + (nt - 1) // 2))
                for b in range(NB):
                    i = nt * NB + b
                    if i < 2:
                        # banks 0/1 freed once the DVE consumed t blocks 14/15
                        tensor.wait_ge(t_sem, 15 + i)
                    elif i >= PSB:
                        tensor.wait_ge(c_sem, i - PSB + 1)
                    for k in range(KT):
                        mm = nc.tensor.matmul(
                            psm[:, i % PSB, 0:NTW],
                            tq[:, b, k],
                            trh[:, nt % NTB, k],
                            start=(k == 0),
                            stop=(k == KT - 1),
                        )
                        if k == KT - 1:
                            mm.then_inc(mm_sem, 1)

        @block.vector
        def _(vector):
            vector.wait_ge(dact, 16)  # eye
            for b in range(NB):
                vector.wait_ge(mm_sem, b + 1)
                nc.vector.tensor_copy(dscr[:], psm[:, b % 2, 0:P])
                nc.vector.scalar_tensor_tensor(
                    out=dscr[:],
                    in0=dscr[:],
                    scalar=1.0,
                    in1=teye[:],
                    op0=ge.mult,
                    op1=ge.mult,
                    accum_out=tall[:, b : b + 1],
                ).then_inc(t_sem, 1)
            for nt in range(NT):
                for b in range(NB):
                    i = nt * NB + b
                    vector.wait_ge(mm_sem, NB + i + 1)
                    nc.vector.tensor_scalar(
                        cscr[:],
                        psm[:, i % PSB, 0:NTW],
                        tall[:, b : b + 1],
                        0.0,
                        op0=ge.is_gt,
                        op1=ge.add,
                        accum_out=acc[:, b, nt : nt + 1],
                    ).then_inc(c_sem, 1)
            for b in range(NB):
                red = nc.vector.tensor_reduce(
                    cnt[:, b : b + 1],
                    acc[:, b],
                    axis=mybir.AxisListType.X,
                    op=ge.add,
                )
                if b == NB - 1:
                    red.then_inc(c_sem, 1)

    return nc


def _build():
    if "nc" not in _CACHE:
        import concourse.mybir as mybir

        _CACHE["mybir"] = mybir
        _CACHE["nc"] = _gen()
    return _CACHE["nc"]


def _run_pjrt(nc, in_maps, n_cores, reps=0):
    """Mirror of bass2jax.run_bass_via_pjrt with device-resident inputs and
    optional repeat timing (no donation so buffers can be reused)."""
    import time as _time

    import jax
    from jax.sharding import Mesh, NamedSharding, PartitionSpec

    try:
        from jax.experimental.shard_map import shard_map
    except ImportError:  # newer jax
        from jax.shard_map import shard_map

    import concourse.mybir as mybir
    from concourse import bass2jax

    bass2jax.install_neuronx_cc_hook()
    partition_name = nc.partition_id_tensor.name if nc.partition_id_tensor else None
    in_names, out_names, out_avals, zero_outs = [], [], [], []
    for alloc in nc.m.functions[0].allocations:
        if not isinstance(alloc, mybir.MemoryLocationSet):
            continue
        name = alloc.memorylocations[0].name
        if alloc.kind == "ExternalInput":
            if name != partition_name:
                in_names.append(name)
        elif alloc.kind == "ExternalOutput":
            out_names.append(name)
            shape = tuple(alloc.tensor_shape)
            dtype = mybir.dt.np(alloc.dtype)
            out_avals.append(jax.core.ShapedArray(shape, dtype))
            zero_outs.append(np.zeros(shape, dtype))
    n_params = len(in_names)
    names_all = in_names + out_names + ([partition_name] if partition_name else [])

    def _body(*args):
        operands = list(args)
        if partition_name:
            operands.append(bass2jax.partition_id_tensor())
        outs = bass2jax._bass_exec_p.bind(
            *operands,
            out_avals=tuple(out_avals),
            in_names=tuple(names_all),
            out_names=tuple(out_names),
            lowering_input_output_aliases=(),
            sim_require_finite=True,
            sim_require_nnan=True,
            nc=nc,
        )
        return tuple(outs)

    devices = jax.devices()[:n_cores]
    mesh = Mesh(np.asarray(devices), ("core",))
    in_specs = (PartitionSpec("core"),) * (n_params + len(out_names))
    out_specs = (PartitionSpec("core"),) * len(out_names)
    fn = jax.jit(
        shard_map(
            _body, mesh=mesh, in_specs=in_specs, out_specs=out_specs, check_rep=False
        ),
        keep_unused=True,
    )
    concat_in = [
        np.concatenate([np.asarray(in_maps[c][nm]) for c in range(n_cores)], axis=0)
        for nm in in_names
    ]
    concat_zeros = [
        np.zeros((n_cores * z.shape[0], *z.shape[1:]), z.dtype) for z in zero_outs
    ]
    sh = NamedSharding(mesh, PartitionSpec("core"))
    dev_in = [jax.device_put(x, sh) for x in concat_in]
    dev_zero = [jax.device_put(x, sh) for x in concat_zeros]
    out = fn(*dev_in, *dev_zero)
    jax.block_until_ready(out)
    times = []
    for _ in range(reps):
        t0 = _time.perf_counter()
        o = fn(*dev_in, *dev_zero)
        jax.block_until_ready(o)
        times.append(_time.perf_counter() - t0)
    results = [
        {
            name: np.asarray(out[i]).reshape(n_cores, *out_avals[i].shape)[c]
            for i, name in enumerate(out_names)
        }
        for c in range(n_cores)
    ]
    return results, (min(times) if times else None)


def _run_device(qT, rhs, gt, eye, trace=False, reps=0):
    nc = _build()
    in_maps = []
    for c in range(NCORES):
        in_maps.append(
            {
                "qT": qT,
                "rhsc": np.ascontiguousarray(
                    rhs[:, c * COLS : (c + 1) * COLS]
                    .reshape(KT, P, NT, NTW)
                    .transpose(2, 1, 0, 3)
                ),
                "gt": gt,
                "eye": eye,
            }
        )
    return _run_pjrt(nc, in_maps, NCORES, reps=reps)


def kernel(q, rhs, queries, filter_idx, _trace=False, _ret_exec=False, _reps=0):
    q = np.asarray(q, dtype=np.float32)
    rhs = np.asarray(rhs, dtype=np.float32)
    true_rhs = np.asarray(queries)[:, 2].astype(np.int64)
    filt = np.asarray(filter_idx).astype(np.int64)

    # [P, NB, KT, P]: [p, b, k, j] = q[b*128+j, k*128+p]
    qT = np.ascontiguousarray(q.T.reshape(KT, P, NB, P).transpose(1, 2, 0, 3))
    gt = np.ascontiguousarray(
        rhs[:, true_rhs].reshape(KT, P, NB, P).transpose(1, 2, 0, 3)
    )
    eye = np.eye(P, dtype=np.float32)

    results, exec_s = _run_device(qT, rhs, gt, eye, reps=_reps)

    counts = np.zeros(B, dtype=np.float64)
    for c in range(NCORES):
        cc = results[c]["cnt"]  # [P, NB]
        counts += cc.T.reshape(B)  # row b*128+p = cc[p, b]
    t = results[0]["tv"].T.reshape(B).astype(np.float32)  # device true scores

    # host correction: dedupe filter, drop entries equal to true tail
    q64 = q.astype(np.float64)
    corr = np.zeros(B, dtype=np.float64)
    CH = 256
    for s in range(0, B, CH):
        e = s + CH
        idx = filt[s:e]  # [CH, 64]
        cols = rhs[:, idx.reshape(-1)].astype(np.float64)  # [512, CH*64]
        sc = np.einsum(
            "bd,dbf->bf", q64[s:e], cols.reshape(D, e - s, idx.shape[1])
        )  # [CH, 64]
        gtmask = sc > t[s:e, None].astype(np.float64)
        # dedupe within row + exclude true index
        srt = np.sort(idx, axis=1)
        first = np.ones_like(idx, dtype=bool)
        order = np.argsort(idx, axis=1, kind="stable")
        dup = srt[:, 1:] == srt[:, :-1]
        fsorted = np.ones_like(idx, dtype=bool)
        fsorted[:, 1:] = ~dup
        np.put_along_axis(first, order, fsorted, axis=1)
        valid = first & (idx != true_rhs[s:e, None])
        corr[s:e] = (gtmask & valid).sum(axis=1)

    ranks = 1.0 + counts - corr
    ranks = np.maximum(ranks, 1.0).astype(np.float32)
    if _ret_exec:
        return ranks, exec_s
    return ranks
